# revision 33
# baseline (speedup 1.0000x reference)
"""Trainium2 Bass kernel for nn_BandwidthPredictorNNHall.

Math: for each batch b (8 of them, one per NeuronCore) with particles
x [n=1024, d=4]:
    pilot_d = 1.0592 * std(x_d, ddof=1) * n^(-1/8)
    q = x / pilot,   K_ij = exp(-0.5 * |q_i - q_j|^2)
    s2_d = sum_ij K_ij ((q_jd - q_id)^2 - 1)
    s3 terms are exactly 0 by antisymmetry -> bandwidth2 treated as 0.

K is symmetric, so only the upper-triangular block half is computed:
row tile ir (128 rows) covers columns [128*ir, 1024).  Diagonal blocks
are half-weighted by accumulating -ln2 into their Gram PSUM region (a
tiny bf16 rank-1 matmul) so that, with
    PT[a, j] = sum_{i in tiles <= tile(j)} Mp[i, a] K''_ij
(K'' = K * e^{r_j/2}, the column factor from the bias-only exp), the
host gets W = PT @ (Mp * e^{-r/2}) and V = W + W^T == Mp^T K Mp
exactly.  The device ships PT [9, 1024] and pilot^2; the ~0.6 Mflop W
assembly and the final ~30 scalar flops run on the host.

Device pipeline per core:
  - One DMA in: particle-major [128, 8(tile), 4].
  - Stats on PE ([4, {1,2}] PSUM columns, pre-scaled ones vectors so the
    DVE var chain is copy/mult/sub/reciprocal -> phcol2 = 1/pilot^2,
    denf = pilot^2).
  - 8 PE transposes build raw q^T in PSUM; ScalarE Copy activations with
    per-partition scale phcol2 produce the scaled Gram weights qsc; the
    raw moving operand qraw = qsc un-scaled by denf (one DVE
    tensor_scalar per half; f32r-rounded producers as fp32r requires).
  - exp bias -r/2: PE transpose + rank-1 broadcast of 0.5/pilot^2,
    Pool multiply, DVE reduce.
  - Main loop (ascending tiles): Gram chunks (f32r, 1 cycle/col) ->
    one Exp per tile (bias -r_i/2) -> bf16 KT -> PT chunks (bf16)
    accumulating into a nested [9, 1024] PSUM region.  PSUM column
    block ir is final right after tile ir, so PT is staged out to SBUF
    in three pieces (after tiles 3, 6, 7) and the tail is just the last
    [9, 128] copy + one DMA.
"""

import sys

sys.path.insert(0, "/opt/trn_rl_repo")

import numpy as np

_B, _N, _D = 8, 1024, 4
_P = 128
_NT = _N // _P  # 8 row tiles
_NM = 1 + 2 * _D  # 9 basis columns: [1, p, p^2]
_INV_SQRT_2PI = 1.0 / np.sqrt(2.0 * np.pi)
_RK = 0.282095
_FACT = 1.0592 * float(_N) ** (-1.0 / (4 + _D))

# column extents per row tile (upper triangle) and KT storage offsets
_C = [(_N - _P * ir) for ir in range(_NT)]
_OFF = [sum(_C[:ir]) for ir in range(_NT)]
_KTOT = sum(_C)  # 4608

_NC = None  # compiled Bass module cache


def _build_kernel():
    import concourse.bass as bass  # noqa: F401
    import concourse.tile as tile
    from concourse import bacc, mybir
    from concourse.masks import make_identity

    f32 = mybir.dt.float32
    fr = mybir.dt.float32r
    bf16 = mybir.dt.bfloat16
    Act = mybir.ActivationFunctionType
    Alu = mybir.AluOpType
    Ax = mybir.AxisListType

    # split -ln2 across the two bf16 rank-1 factors so the product is
    # ln2 to ~2^-16 relative
    import ml_dtypes

    _lx = float(np.abs(np.sqrt(np.log(2.0))).astype(ml_dtypes.bfloat16))
    _ly = float(np.array(np.log(2.0) / _lx, np.float32).astype(ml_dtypes.bfloat16))

    nc = bacc.Bacc("TRN2", target_bir_lowering=False, debug=False, num_devices=_B)
    p_in = nc.dram_tensor("p", [_N, _D], f32, kind="ExternalInput")
    w_out = nc.dram_tensor("wout", [_NM, _N + 16], f32, kind="ExternalOutput")

    with tile.TileContext(nc) as tc:
        with (
            tc.tile_pool(name="singles", bufs=1) as singles,
            tc.tile_pool(name="psG", bufs=1, space="PSUM") as psG,
            tc.tile_pool(name="psPT", bufs=1, space="PSUM") as psPT,
        ):
            ident128 = singles.tile([_P, _P], f32, tag="identf")
            make_identity(nc, ident128)
            ones128 = singles.tile([_P, 1], f32, tag="ones128")
            nc.vector.memset(ones128, 1.0)
            # pre-scaled ones so denf comes out as pilot^2 directly:
            #   m = (sum p * a)^2, denf = sum p^2 * b - m
            bconst = _FACT * _FACT / (_N - 1)
            aconst = _FACT / np.sqrt(float(_N) * (_N - 1))
            onesA = singles.tile([_P, 1], f32, tag="onesA")
            nc.vector.memset(onesA, aconst)
            onesB = singles.tile([_P, 1], f32, tag="onesB")
            nc.vector.memset(onesB, bconst)
            halfrow = singles.tile([1, _P], f32, tag="halfrow")
            nc.vector.memset(halfrow, 0.5)
            # bf16 rank-1 factors for the -ln2 diagonal half-weighting
            lnrow = singles.tile([1, _P], bf16, tag="lnrow")
            nc.vector.memset(lnrow, -_lx)
            lyrow = singles.tile([1, _P], bf16, tag="lyrow")
            nc.vector.memset(lyrow, _ly)
            # dummy Exp so the activation-table load runs during the DMA wait
            warm = singles.tile([1, 1], f32, tag="warm")
            nc.scalar.activation(out=warm, in_=ones128[0:1, 0:1], func=Act.Exp)

            # ---- input DMA: particle-major tiles
            mstatall = singles.tile([_P, _NT, _D], f32, tag="mstatall")
            nc.sync.dma_start(
                out=mstatall,
                in_=p_in[:].rearrange("(c i) d -> i c d", c=_NT),
            )

            msqall = singles.tile([_P, _NT, _D], f32, tag="msqall")
            nc.vector.tensor_mul(msqall, mstatall, mstatall)

            # ---- small PSUM staging lives in gram-ring slot 0
            # (stats cols 0/2, row-stats 8:16, denf row 20:24, bc 32:36)
            smallA = psG.tile([_P, 512], f32, tag="g", bufs=3, name="smallA")
            st8 = smallA[0:_D, 0:3]
            for c in range(_NT):
                nc.tensor.matmul(
                    st8[:, 0:1], lhsT=mstatall[:, c, :], rhs=onesA,
                    start=(c == 0), stop=(c == _NT - 1),
                    skip_group_check=True,
                )
            for c in range(_NT):
                nc.tensor.matmul(
                    st8[:, 2:3], lhsT=msqall[:, c, :], rhs=onesB,
                    start=(c == 0), stop=(c == _NT - 1),
                    skip_group_check=True,
                )
            # row-form stats (partition 0) for the exp-bias broadcast:
            # the rank-1 rhs needs 1/pilot^2 as a row without a transpose
            strow = smallA[0:1, 8:16]
            for c in range(_NT):
                nc.tensor.matmul(
                    strow[:, 0:_D], lhsT=onesA, rhs=mstatall[:, c, :],
                    start=(c == 0), stop=(c == _NT - 1),
                    skip_group_check=True,
                )
            for c in range(_NT):
                nc.tensor.matmul(
                    strow[:, _D : 2 * _D], lhsT=onesB, rhs=msqall[:, c, :],
                    start=(c == 0), stop=(c == _NT - 1),
                    skip_group_check=True,
                )

            # ---- 8 PE transposes: raw q^T into PSUM [4, 1024]
            # (gram-ring slot 1; reused by gram tile 2)
            psq = psG.tile([_D, _N], f32, tag="g", bufs=3)
            for c in range(_NT):
                nc.tensor.transpose(
                    psq[:, c * _P : (c + 1) * _P], mstatall[:, c, :], ident128
                )

            # ---- var chains: squares on ScalarE (idle), sub/recip on
            # DVE.  Column form (per-partition scales) + row form (rank-1
            # broadcast rhs).
            mm = singles.tile([_D, 1], f32, tag="mm")
            nc.scalar.square(mm, st8[:, 0:1])
            mmr = singles.tile([1, _D], f32, tag="mmr")
            nc.scalar.square(mmr, strow[:, 0:_D])
            denf = singles.tile([_D, 1], f32, tag="denf")
            nc.vector.tensor_sub(denf, st8[:, 2:3], mm)
            phcol2 = singles.tile([_D, 1], f32, tag="phcol2")
            nc.vector.reciprocal(phcol2, denf)
            denfr = singles.tile([1, _D], f32, tag="denfr")
            nc.vector.tensor_sub(denfr, strow[:, _D : 2 * _D], mmr)
            phrow = singles.tile([1, _D], f32, tag="phrow")
            nc.vector.reciprocal(phrow, denfr)

            # ---- scaled weights qsc = q^T * phcol2 on ScalarE (idle
            # until the first exp); halves so qraw can chase
            qsc = singles.tile([_D, _N], fr, tag="qsc")
            nc.scalar.activation(
                out=qsc[:, 0:512], in_=psq[:, 0:512], func=Act.Copy, scale=phcol2
            )
            nc.scalar.activation(
                out=qsc[:, 512:_N], in_=psq[:, 512:_N], func=Act.Copy, scale=phcol2
            )

            # ---- tile-0 Gram weights: qraw slice 0 = qsc[:, 0:128]
            # un-scaled by denf (exact up to the reciprocal rounding)
            qraw = singles.tile([_D, _N], fr, tag="qraw")
            nc.vector.tensor_scalar_mul(qraw[:, 0:_P], qsc[:, 0:_P], denf)

            # ---- 0.5/pilot^2 broadcast [128, 4] via one PE rank-1
            bc_ps = smallA[:, 32:36]
            nc.tensor.matmul(
                bc_ps, lhsT=halfrow, rhs=phrow,
                start=True, stop=True, skip_group_check=True,
            )
            bc_sb = singles.tile([_P, _D], f32, tag="bc_sb")
            nc.vector.tensor_copy(bc_sb, bc_ps)

            # ---- exp bias nhall[:, c] = -r/2
            scr = singles.tile([_P, _NT, _D], f32, tag="scr")
            nc.vector.tensor_mul(
                scr, msqall,
                bc_sb.rearrange("p (o d) -> p o d", o=1).broadcast_to([_P, _NT, _D]),
            )
            nhall = singles.tile([_P, _NT], f32, tag="nhall")
            nc.vector.tensor_reduce(
                out=nhall.rearrange("p (c o) -> p c o", o=1), in_=scr,
                axis=Ax.X, op=Alu.add, negate=True,
            )

            # pilot^2 row for the output staging (smallA dies before the
            # gram ring reuses its slot)
            denf_ps = smallA[0:1, 20:24]
            nc.tensor.matmul(
                denf_ps, lhsT=denf, rhs=ident128[0:_D, 0:_D],
                is_transpose=True, skip_group_check=True,
            )

            # ---- Mp in bf16 (PT-stage weights)
            mtall = singles.tile([_P, _NT, _NM], bf16, tag="mtall")
            nc.vector.memset(mtall[:, :, 0:1], 1.0)
            nc.vector.tensor_copy(mtall[:, :, 1 : 1 + _D], mstatall)
            nc.vector.tensor_copy(mtall[:, :, 1 + _D : _NM], msqall)

            # ---- main loop: upper-triangle row tiles, ascending
            ktall = singles.tile([_P, _KTOT], bf16, tag="ktall")
            pspt = psPT.tile([_NM, _N], f32, tag="pspt")
            pt_sb = singles.tile([_NM, _N + 16], f32, tag="pt_sb")

            def gram(ir):
                lo = _P * ir
                g = psG.tile([_P, _N], f32, tag="g", bufs=3, name=f"g{ir}")
                w = qraw[:, lo : lo + _P]
                if lo < 512:
                    nc.tensor.matmul(
                        g[:, lo:512], lhsT=w, rhs=qsc[:, lo:512],
                        start=True, stop=False, skip_group_check=True,
                    )
                    # -ln2 into the diagonal block (half-weighting); before
                    # chunk B so the chunk-A exp never waits on B
                    nc.tensor.matmul(
                        g[:, lo : lo + _P], lhsT=lnrow, rhs=lyrow,
                        start=False, stop=True, skip_group_check=True,
                    )
                    nc.tensor.matmul(
                        g[:, 512:_N], lhsT=w, rhs=qsc[:, 512:_N],
                        start=True, stop=True, skip_group_check=True,
                    )
                else:
                    nc.tensor.matmul(
                        g[:, lo:_N], lhsT=w, rhs=qsc[:, lo:_N],
                        start=True, stop=False, skip_group_check=True,
                    )
                    nc.tensor.matmul(
                        g[:, lo : lo + _P], lhsT=lnrow, rhs=lyrow,
                        start=False, stop=True, skip_group_check=True,
                    )
                return g

            def exp_tile(ir, g):
                lo = _P * ir
                if ir == 0:
                    nc.scalar.activation(
                        out=ktall[:, 0:512], in_=g[:, 0:512],
                        func=Act.Exp, bias=nhall[:, 0:1],
                    )
                    nc.scalar.activation(
                        out=ktall[:, 512:_N], in_=g[:, 512:_N],
                        func=Act.Exp, bias=nhall[:, 0:1],
                    )
                else:
                    nc.scalar.activation(
                        out=ktall[:, _OFF[ir] : _OFF[ir] + _C[ir]],
                        in_=g[:, lo:_N],
                        func=Act.Exp, bias=nhall[:, ir : ir + 1],
                    )

            def pt_tile(ir):
                lo = _P * ir
                off = _OFF[ir]
                if lo < 512:
                    nc.tensor.matmul(
                        pspt[:, lo:512], lhsT=mtall[:, ir, :],
                        rhs=ktall[:, off : off + (512 - lo)],
                        start=(ir == 0), stop=(ir == 3),
                        skip_group_check=True,
                    )
                    nc.tensor.matmul(
                        pspt[:, 512:_N], lhsT=mtall[:, ir, :],
                        rhs=ktall[:, off + (512 - lo) : off + _C[ir]],
                        start=(ir == 0), stop=(ir == _NT - 1),
                        skip_group_check=True,
                    )
                else:
                    nc.tensor.matmul(
                        pspt[:, lo:_N], lhsT=mtall[:, ir, :],
                        rhs=ktall[:, off : off + _C[ir]],
                        start=False, stop=(ir == _NT - 1),
                        skip_group_check=True,
                    )

            def qraw_slice(k):
                nc.vector.tensor_scalar_mul(
                    qraw[:, _P * k : _P * (k + 1)],
                    qsc[:, _P * k : _P * (k + 1)], denf,
                )

            qraw_slice(1)
            gs = [gram(0), gram(1)]
            nc.vector.tensor_copy(pt_sb[0:1, _N : _N + _D], denf_ps)
            for ir in range(_NT):
                exp_tile(ir, gs[ir])
                if ir + 2 < _NT:
                    qraw_slice(ir + 2)
                    gs.append(gram(ir + 2))
                pt_tile(ir)
                # stage finished PT column blocks out to SBUF
                if ir == 3:
                    nc.vector.tensor_copy(pt_sb[:, 0:512], pspt[:, 0:512])
                elif ir == 5:
                    nc.vector.tensor_copy(pt_sb[:, 512:768], pspt[:, 512:768])
                elif ir == 6:
                    nc.vector.tensor_copy(pt_sb[:, 768:896], pspt[:, 768:896])

            nc.vector.tensor_copy(pt_sb[:, 896:_N], pspt[:, 896:_N])

            nc.gpsimd.dma_start(out=w_out[:], in_=pt_sb)

    nc.compile()
    return nc


def _get_nc():
    global _NC
    if _NC is None:
        _NC = _build_kernel()
    return _NC


def finalize(w, p):
    """Host-side tail.  w [9, 1040]: cols 0:1024 = PT (upper-tri half,
    diag blocks half-weighted, column factor e^{+r_j/2}), row 0 cols
    1024:1028 = pilot^2.  p [1024, 4] = this batch's particles."""
    PT = w[:, 0:_N].astype(np.float64)
    pilot2 = w[0, _N : _N + _D].astype(np.float64)
    p = p.astype(np.float64)
    r = (p * p / pilot2[None, :]).sum(axis=1)
    cx = np.exp(-0.5 * r)
    Mp = np.concatenate([np.ones((_N, 1)), p, p * p], axis=1)
    W = PT @ (Mp * cx[:, None])  # [9, 9]
    d = np.arange(_D)
    g = 2.0 * (W[0, 5 + d] + W[5 + d, 0] - 2.0 * W[1 + d, 1 + d])
    v00 = 2.0 * W[0, 0]
    s2 = (g / pilot2 - v00) * _INV_SQRT_2PI
    denom = _N * (_N - 1)
    pilot5 = pilot2**2 * np.sqrt(pilot2)
    I2 = s2 / pilot5 / denom
    J1 = _RK / I2
    base = J1 / _N
    return (np.sign(base) * np.abs(base) ** 0.2).astype(np.float32)


def kernel(particles, weights=None, **_unused):
    from concourse.bass_utils import run_bass_kernel_spmd

    particles = np.ascontiguousarray(np.asarray(particles), dtype=np.float32)
    assert particles.shape == (_B, _N, _D), particles.shape

    nc = _get_nc()
    in_maps = [{"p": particles[c]} for c in range(_B)]
    res = run_bass_kernel_spmd(nc, in_maps, list(range(_B)))

    out = np.empty((_B, _D), np.float32)
    for c in range(_B):
        out[c] = finalize(res.results[c]["wout"], particles[c])
    return out


# revision 34
# speedup vs baseline: 1.0273x; 1.0273x over previous
"""Trainium2 Bass kernel for nn_BandwidthPredictorNNHall.

Math: for each batch b (8 of them, one per NeuronCore) with particles
x [n=1024, d=4]:
    pilot_d = 1.0592 * std(x_d, ddof=1) * n^(-1/8)
    q = x / pilot,   K_ij = exp(-0.5 * |q_i - q_j|^2)
    s2_d = sum_ij K_ij ((q_jd - q_id)^2 - 1)
    s3 terms are exactly 0 by antisymmetry -> bandwidth2 treated as 0.

K is symmetric, so only the upper-triangular block half is computed:
row tile ir (128 rows) covers columns [128*ir, 1024).  Diagonal blocks
are half-weighted by accumulating -ln2 into their Gram PSUM region (a
tiny bf16 rank-1 matmul) so that, with
    PT[a, j] = sum_{i in tiles <= tile(j)} Mp[i, a] K''_ij
(K'' = K * e^{r_j/2}, the column factor from the bias-only exp), the
host gets W = PT @ (Mp * e^{-r/2}) and V = W + W^T == Mp^T K Mp
exactly.  The device ships PT [9, 1024] and pilot^2; the ~0.6 Mflop W
assembly and the final ~30 scalar flops run on the host.

Device pipeline per core:
  - One DMA in: particle-major [128, 8(tile), 4].
  - Stats on PE ([4, {1,2}] PSUM columns, pre-scaled ones vectors so the
    DVE var chain is copy/mult/sub/reciprocal -> phcol2 = 1/pilot^2,
    denf = pilot^2).
  - 8 PE transposes build raw q^T in PSUM; ScalarE Copy activations with
    per-partition scale phcol2 produce the scaled Gram weights qsc; the
    raw moving operand qraw = qsc un-scaled by denf (one DVE
    tensor_scalar per half; f32r-rounded producers as fp32r requires).
  - exp bias -r/2: PE transpose + rank-1 broadcast of 0.5/pilot^2,
    Pool multiply, DVE reduce.
  - Main loop (ascending tiles): Gram chunks (f32r, 1 cycle/col) ->
    one Exp per tile (bias -r_i/2) -> bf16 KT -> PT chunks (bf16)
    accumulating into a nested [9, 1024] PSUM region.  PSUM column
    block ir is final right after tile ir, so PT is staged out to SBUF
    in three pieces (after tiles 3, 6, 7) and the tail is just the last
    [9, 128] copy + one DMA.
"""

import sys

sys.path.insert(0, "/opt/trn_rl_repo")

import numpy as np

_B, _N, _D = 8, 1024, 4
_P = 128
_NT = _N // _P  # 8 row tiles
_NM = 1 + 2 * _D  # 9 basis columns: [1, p, p^2]
_INV_SQRT_2PI = 1.0 / np.sqrt(2.0 * np.pi)
_RK = 0.282095
_FACT = 1.0592 * float(_N) ** (-1.0 / (4 + _D))

# column extents per row tile (upper triangle) and KT storage offsets
_C = [(_N - _P * ir) for ir in range(_NT)]
_OFF = [sum(_C[:ir]) for ir in range(_NT)]
_KTOT = sum(_C)  # 4608

_NC = None  # compiled Bass module cache


def _build_kernel():
    import concourse.bass as bass  # noqa: F401
    import concourse.tile as tile
    from concourse import bacc, mybir
    from concourse.masks import make_identity

    f32 = mybir.dt.float32
    fr = mybir.dt.float32r
    bf16 = mybir.dt.bfloat16
    Act = mybir.ActivationFunctionType
    Alu = mybir.AluOpType
    Ax = mybir.AxisListType

    # split -ln2 across the two bf16 rank-1 factors so the product is
    # ln2 to ~2^-16 relative
    import ml_dtypes

    _lx = float(np.abs(np.sqrt(np.log(2.0))).astype(ml_dtypes.bfloat16))
    _ly = float(np.array(np.log(2.0) / _lx, np.float32).astype(ml_dtypes.bfloat16))

    nc = bacc.Bacc("TRN2", target_bir_lowering=False, debug=False, num_devices=_B)
    p_in = nc.dram_tensor("p", [_N, _D], f32, kind="ExternalInput")
    w_out = nc.dram_tensor("wout", [_NM, _N + 16], f32, kind="ExternalOutput")

    with tile.TileContext(nc) as tc:
        with (
            tc.tile_pool(name="singles", bufs=1) as singles,
            tc.tile_pool(name="psG", bufs=1, space="PSUM") as psG,
            tc.tile_pool(name="psPT", bufs=1, space="PSUM") as psPT,
        ):
            ident128 = singles.tile([_P, _P], f32, tag="identf")
            make_identity(nc, ident128)
            ones128 = singles.tile([_P, 1], f32, tag="ones128")
            nc.vector.memset(ones128, 1.0)
            # pre-scaled ones so denf comes out as pilot^2 directly:
            #   m = (sum p * a)^2, denf = sum p^2 * b - m
            bconst = _FACT * _FACT / (_N - 1)
            aconst = _FACT / np.sqrt(float(_N) * (_N - 1))
            onesA = singles.tile([_P, 1], f32, tag="onesA")
            nc.vector.memset(onesA, aconst)
            onesB = singles.tile([_P, 1], f32, tag="onesB")
            nc.vector.memset(onesB, bconst)
            halfrow = singles.tile([1, _P], f32, tag="halfrow")
            nc.vector.memset(halfrow, 0.5)
            # bf16 rank-1 factors for the -ln2 diagonal half-weighting
            lnrow = singles.tile([1, _P], bf16, tag="lnrow")
            nc.vector.memset(lnrow, -_lx)
            lyrow = singles.tile([1, _P], bf16, tag="lyrow")
            nc.vector.memset(lyrow, _ly)
            # dummy Exp so the activation-table load runs during the DMA wait
            warm = singles.tile([1, 1], f32, tag="warm")
            nc.scalar.activation(out=warm, in_=ones128[0:1, 0:1], func=Act.Exp)

            # ---- input DMA: particle-major tiles
            mstatall = singles.tile([_P, _NT, _D], f32, tag="mstatall")
            nc.sync.dma_start(
                out=mstatall,
                in_=p_in[:].rearrange("(c i) d -> i c d", c=_NT),
            )

            msqall = singles.tile([_P, _NT, _D], f32, tag="msqall")
            nc.vector.tensor_mul(msqall, mstatall, mstatall)

            # ---- small PSUM staging lives in gram-ring slot 0
            # (stats cols 0/2, row-stats 8:16, denf row 20:24, bc 32:36)
            smallA = psG.tile([_P, 512], f32, tag="g", bufs=3, name="smallA")
            st8 = smallA[0:_D, 0:3]
            for c in range(_NT):
                nc.tensor.matmul(
                    st8[:, 0:1], lhsT=mstatall[:, c, :], rhs=onesA,
                    start=(c == 0), stop=(c == _NT - 1),
                    skip_group_check=True,
                )
            for c in range(_NT):
                nc.tensor.matmul(
                    st8[:, 2:3], lhsT=msqall[:, c, :], rhs=onesB,
                    start=(c == 0), stop=(c == _NT - 1),
                    skip_group_check=True,
                )
            # row-form stats (partition 0) for the exp-bias broadcast:
            # the rank-1 rhs needs 1/pilot^2 as a row without a transpose
            strow = smallA[0:1, 8:16]
            for c in range(_NT):
                nc.tensor.matmul(
                    strow[:, 0:_D], lhsT=onesA, rhs=mstatall[:, c, :],
                    start=(c == 0), stop=(c == _NT - 1),
                    skip_group_check=True,
                )
            for c in range(_NT):
                nc.tensor.matmul(
                    strow[:, _D : 2 * _D], lhsT=onesB, rhs=msqall[:, c, :],
                    start=(c == 0), stop=(c == _NT - 1),
                    skip_group_check=True,
                )

            # ---- 8 PE transposes: raw q^T into PSUM [4, 1024]
            # (gram-ring slot 1; reused by gram tile 2)
            psq = psG.tile([_D, _N], f32, tag="g", bufs=3)
            for c in range(_NT):
                nc.tensor.transpose(
                    psq[:, c * _P : (c + 1) * _P], mstatall[:, c, :], ident128
                )

            # ---- var chains: squares on ScalarE (idle), sub/recip on
            # DVE.  Column form (per-partition scales) + row form (rank-1
            # broadcast rhs).
            mm = singles.tile([_D, 1], f32, tag="mm")
            nc.scalar.square(mm, st8[:, 0:1])
            mmr = singles.tile([1, _D], f32, tag="mmr")
            nc.scalar.square(mmr, strow[:, 0:_D])
            denf = singles.tile([_D, 1], f32, tag="denf")
            nc.vector.tensor_sub(denf, st8[:, 2:3], mm)
            phcol2 = singles.tile([_D, 1], f32, tag="phcol2")
            nc.vector.reciprocal(phcol2, denf)
            denfr = singles.tile([1, _D], f32, tag="denfr")
            nc.vector.tensor_sub(denfr, strow[:, _D : 2 * _D], mmr)
            phrow = singles.tile([1, _D], f32, tag="phrow")
            nc.vector.reciprocal(phrow, denfr)

            # ---- scaled weights qsc = q^T * phcol2 on ScalarE (idle
            # until the first exp); halves so qraw can chase
            qsc = singles.tile([_D, _N], fr, tag="qsc")
            nc.scalar.activation(
                out=qsc[:, 0:512], in_=psq[:, 0:512], func=Act.Copy, scale=phcol2
            )
            nc.scalar.activation(
                out=qsc[:, 512:_N], in_=psq[:, 512:_N], func=Act.Copy, scale=phcol2
            )

            # ---- tile-0 Gram weights: qraw slice 0 = qsc[:, 0:128]
            # un-scaled by denf (exact up to the reciprocal rounding)
            qraw = singles.tile([_D, _N], fr, tag="qraw")
            nc.vector.tensor_scalar_mul(qraw[:, 0:_P], qsc[:, 0:_P], denf)

            # ---- 0.5/pilot^2 broadcast [128, 4] via one PE rank-1
            bc_ps = smallA[:, 32:36]
            nc.tensor.matmul(
                bc_ps, lhsT=halfrow, rhs=phrow,
                start=True, stop=True, skip_group_check=True,
            )
            bc_sb = singles.tile([_P, _D], f32, tag="bc_sb")
            nc.vector.tensor_copy(bc_sb, bc_ps)

            # ---- exp bias nhall[:, c] = -r/2
            scr = singles.tile([_P, _NT, _D], f32, tag="scr")
            nc.vector.tensor_mul(
                scr, msqall,
                bc_sb.rearrange("p (o d) -> p o d", o=1).broadcast_to([_P, _NT, _D]),
            )
            nhall = singles.tile([_P, _NT], f32, tag="nhall")
            nc.vector.tensor_reduce(
                out=nhall.rearrange("p (c o) -> p c o", o=1), in_=scr,
                axis=Ax.X, op=Alu.add, negate=True,
            )

            # pilot^2 row for the output staging (smallA dies before the
            # gram ring reuses its slot)
            denf_ps = smallA[0:1, 20:24]
            nc.tensor.matmul(
                denf_ps, lhsT=denf, rhs=ident128[0:_D, 0:_D],
                is_transpose=True, skip_group_check=True,
            )

            # ---- Mp in bf16 (PT-stage weights)
            mtall = singles.tile([_P, _NT, _NM], bf16, tag="mtall")
            nc.vector.memset(mtall[:, :, 0:1], 1.0)
            nc.vector.tensor_copy(mtall[:, :, 1 : 1 + _D], mstatall)
            nc.vector.tensor_copy(mtall[:, :, 1 + _D : _NM], msqall)

            # ---- main loop: upper-triangle row tiles, ascending
            ktall = singles.tile([_P, _KTOT], bf16, tag="ktall")
            pspt = psPT.tile([_NM, _N], f32, tag="pspt")
            pt_sb = singles.tile([_NM, _N + 16], f32, tag="pt_sb")

            def gram(ir):
                lo = _P * ir
                g = psG.tile([_P, _N], f32, tag="g", bufs=3, name=f"g{ir}")
                w = qraw[:, lo : lo + _P]
                if lo < 512:
                    nc.tensor.matmul(
                        g[:, lo:512], lhsT=w, rhs=qsc[:, lo:512],
                        start=True, stop=False, skip_group_check=True,
                    )
                    # -ln2 into the diagonal block (half-weighting); before
                    # chunk B so the chunk-A exp never waits on B
                    nc.tensor.matmul(
                        g[:, lo : lo + _P], lhsT=lnrow, rhs=lyrow,
                        start=False, stop=True, skip_group_check=True,
                    )
                    nc.tensor.matmul(
                        g[:, 512:_N], lhsT=w, rhs=qsc[:, 512:_N],
                        start=True, stop=True, skip_group_check=True,
                    )
                else:
                    nc.tensor.matmul(
                        g[:, lo:_N], lhsT=w, rhs=qsc[:, lo:_N],
                        start=True, stop=False, skip_group_check=True,
                    )
                    nc.tensor.matmul(
                        g[:, lo : lo + _P], lhsT=lnrow, rhs=lyrow,
                        start=False, stop=True, skip_group_check=True,
                    )
                return g

            def exp_tile(ir, g):
                lo = _P * ir
                if ir == 0:
                    nc.scalar.activation(
                        out=ktall[:, 0:512], in_=g[:, 0:512],
                        func=Act.Exp, bias=nhall[:, 0:1],
                    )
                    nc.scalar.activation(
                        out=ktall[:, 512:_N], in_=g[:, 512:_N],
                        func=Act.Exp, bias=nhall[:, 0:1],
                    )
                else:
                    nc.scalar.activation(
                        out=ktall[:, _OFF[ir] : _OFF[ir] + _C[ir]],
                        in_=g[:, lo:_N],
                        func=Act.Exp, bias=nhall[:, ir : ir + 1],
                    )

            def pt_tile(ir):
                lo = _P * ir
                off = _OFF[ir]
                if lo < 512:
                    nc.tensor.matmul(
                        pspt[:, lo:512], lhsT=mtall[:, ir, :],
                        rhs=ktall[:, off : off + (512 - lo)],
                        start=(ir == 0), stop=(ir == 3),
                        skip_group_check=True,
                    )
                    nc.tensor.matmul(
                        pspt[:, 512:_N], lhsT=mtall[:, ir, :],
                        rhs=ktall[:, off + (512 - lo) : off + _C[ir]],
                        start=(ir == 0), stop=(ir == _NT - 1),
                        skip_group_check=True,
                    )
                else:
                    nc.tensor.matmul(
                        pspt[:, lo:_N], lhsT=mtall[:, ir, :],
                        rhs=ktall[:, off : off + _C[ir]],
                        start=False, stop=(ir == _NT - 1),
                        skip_group_check=True,
                    )

            def qraw_slice(k):
                nc.vector.tensor_scalar_mul(
                    qraw[:, _P * k : _P * (k + 1)],
                    qsc[:, _P * k : _P * (k + 1)], denf,
                )

            qraw_slice(1)
            gs = [gram(0), gram(1)]
            nc.vector.tensor_copy(pt_sb[0:1, _N : _N + _D], denf_ps)
            for ir in range(_NT):
                exp_tile(ir, gs[ir])
                if ir + 2 < _NT:
                    qraw_slice(ir + 2)
                    gs.append(gram(ir + 2))
                pt_tile(ir)
                # stage finished PT column blocks out to SBUF
                if ir == 3:
                    nc.vector.tensor_copy(pt_sb[:, 0:512], pspt[:, 0:512])
                elif ir == 5:
                    nc.vector.tensor_copy(pt_sb[:, 512:768], pspt[:, 512:768])
                elif ir == 6:
                    nc.vector.tensor_copy(pt_sb[:, 768:896], pspt[:, 768:896])

            nc.vector.tensor_copy(pt_sb[:, 896:_N], pspt[:, 896:_N])

            nc.sync.dma_start(out=w_out[:], in_=pt_sb)

    nc.compile()
    return nc


def _get_nc():
    global _NC
    if _NC is None:
        _NC = _build_kernel()
    return _NC


def finalize(w, p):
    """Host-side tail.  w [9, 1040]: cols 0:1024 = PT (upper-tri half,
    diag blocks half-weighted, column factor e^{+r_j/2}), row 0 cols
    1024:1028 = pilot^2.  p [1024, 4] = this batch's particles."""
    PT = w[:, 0:_N].astype(np.float64)
    pilot2 = w[0, _N : _N + _D].astype(np.float64)
    p = p.astype(np.float64)
    r = (p * p / pilot2[None, :]).sum(axis=1)
    cx = np.exp(-0.5 * r)
    Mp = np.concatenate([np.ones((_N, 1)), p, p * p], axis=1)
    W = PT @ (Mp * cx[:, None])  # [9, 9]
    d = np.arange(_D)
    g = 2.0 * (W[0, 5 + d] + W[5 + d, 0] - 2.0 * W[1 + d, 1 + d])
    v00 = 2.0 * W[0, 0]
    s2 = (g / pilot2 - v00) * _INV_SQRT_2PI
    denom = _N * (_N - 1)
    pilot5 = pilot2**2 * np.sqrt(pilot2)
    I2 = s2 / pilot5 / denom
    J1 = _RK / I2
    base = J1 / _N
    return (np.sign(base) * np.abs(base) ** 0.2).astype(np.float32)


def kernel(particles, weights=None, **_unused):
    from concourse.bass_utils import run_bass_kernel_spmd

    particles = np.ascontiguousarray(np.asarray(particles), dtype=np.float32)
    assert particles.shape == (_B, _N, _D), particles.shape

    nc = _get_nc()
    in_maps = [{"p": particles[c]} for c in range(_B)]
    res = run_bass_kernel_spmd(nc, in_maps, list(range(_B)))

    out = np.empty((_B, _D), np.float32)
    for c in range(_B):
        out[c] = finalize(res.results[c]["wout"], particles[c])
    return out


# revision 35
# speedup vs baseline: 1.0389x; 1.0113x over previous
"""Trainium2 Bass kernel for nn_BandwidthPredictorNNHall.

Math: for each batch b (8 of them, one per NeuronCore) with particles
x [n=1024, d=4]:
    pilot_d = 1.0592 * std(x_d, ddof=1) * n^(-1/8)
    q = x / pilot,   K_ij = exp(-0.5 * |q_i - q_j|^2)
    s2_d = sum_ij K_ij ((q_jd - q_id)^2 - 1)
    s3 terms are exactly 0 by antisymmetry -> bandwidth2 treated as 0.

K is symmetric, so only the upper-triangular block half is computed:
row tile ir (128 rows) covers columns [128*ir, 1024).  Diagonal blocks
are half-weighted by accumulating -ln2 into their Gram PSUM region (a
tiny bf16 rank-1 matmul) so that, with
    PT[a, j] = sum_{i in tiles <= tile(j)} Mp[i, a] K''_ij
(K'' = K * e^{r_j/2}, the column factor from the bias-only exp), the
host gets W = PT @ (Mp * e^{-r/2}) and V = W + W^T == Mp^T K Mp
exactly.  The device ships PT [9, 1024] and pilot^2; the ~0.6 Mflop W
assembly and the final ~30 scalar flops run on the host.

Device pipeline per core:
  - One DMA in: particle-major [128, 8(tile), 4].
  - Stats on PE ([4, {1,2}] PSUM columns, pre-scaled ones vectors so the
    DVE var chain is copy/mult/sub/reciprocal -> phcol2 = 1/pilot^2,
    denf = pilot^2).
  - 8 PE transposes build raw q^T in PSUM; ScalarE Copy activations with
    per-partition scale phcol2 produce the scaled Gram weights qsc; the
    raw moving operand qraw = qsc un-scaled by denf (one DVE
    tensor_scalar per half; f32r-rounded producers as fp32r requires).
  - exp bias -r/2: PE transpose + rank-1 broadcast of 0.5/pilot^2,
    Pool multiply, DVE reduce.
  - Main loop (ascending tiles): Gram chunks (f32r, 1 cycle/col) ->
    one Exp per tile (bias -r_i/2) -> bf16 KT -> PT chunks (bf16)
    accumulating into a nested [9, 1024] PSUM region.  PSUM column
    block ir is final right after tile ir, so PT is staged out to SBUF
    in three pieces (after tiles 3, 6, 7) and the tail is just the last
    [9, 128] copy + one DMA.
"""

import sys

sys.path.insert(0, "/opt/trn_rl_repo")

import numpy as np

_B, _N, _D = 8, 1024, 4
_P = 128
_NT = _N // _P  # 8 row tiles
_NM = 1 + 2 * _D  # 9 basis columns: [1, p, p^2]
_INV_SQRT_2PI = 1.0 / np.sqrt(2.0 * np.pi)
_RK = 0.282095
_FACT = 1.0592 * float(_N) ** (-1.0 / (4 + _D))

# column extents per row tile (upper triangle) and KT storage offsets
_C = [(_N - _P * ir) for ir in range(_NT)]
_OFF = [sum(_C[:ir]) for ir in range(_NT)]
_KTOT = sum(_C)  # 4608

_NC = None  # compiled Bass module cache


def _build_kernel():
    import concourse.bass as bass  # noqa: F401
    import concourse.tile as tile
    from concourse import bacc, mybir
    from concourse.masks import make_identity

    f32 = mybir.dt.float32
    fr = mybir.dt.float32r
    bf16 = mybir.dt.bfloat16
    Act = mybir.ActivationFunctionType
    Alu = mybir.AluOpType
    Ax = mybir.AxisListType

    # split -ln2 across the two bf16 rank-1 factors so the product is
    # ln2 to ~2^-16 relative
    import ml_dtypes

    _lx = float(np.abs(np.sqrt(np.log(2.0))).astype(ml_dtypes.bfloat16))
    _ly = float(np.array(np.log(2.0) / _lx, np.float32).astype(ml_dtypes.bfloat16))

    nc = bacc.Bacc("TRN2", target_bir_lowering=False, debug=False, num_devices=_B)
    p_in = nc.dram_tensor("p", [_N, _D], f32, kind="ExternalInput")
    w_out = nc.dram_tensor("wout", [_NM, _N + 16], f32, kind="ExternalOutput")

    with tile.TileContext(nc) as tc:
        with (
            tc.tile_pool(name="singles", bufs=1) as singles,
            tc.tile_pool(name="psG", bufs=1, space="PSUM") as psG,
            tc.tile_pool(name="psPT", bufs=1, space="PSUM") as psPT,
        ):
            ident128 = singles.tile([_P, _P], f32, tag="identf")
            make_identity(nc, ident128)
            ones128 = singles.tile([_P, 1], f32, tag="ones128")
            nc.vector.memset(ones128, 1.0)
            # pre-scaled ones so denf comes out as pilot^2 directly:
            #   m = (sum p * a)^2, denf = sum p^2 * b - m
            bconst = _FACT * _FACT / (_N - 1)
            aconst = _FACT / np.sqrt(float(_N) * (_N - 1))
            onesA = singles.tile([_P, 1], f32, tag="onesA")
            nc.vector.memset(onesA, aconst)
            onesB = singles.tile([_P, 1], f32, tag="onesB")
            nc.vector.memset(onesB, bconst)
            halfrow = singles.tile([1, _P], f32, tag="halfrow")
            nc.vector.memset(halfrow, 0.5)
            # bf16 rank-1 factors for the -ln2 diagonal half-weighting
            lnrow = singles.tile([1, _P], bf16, tag="lnrow")
            nc.vector.memset(lnrow, -_lx)
            lyrow = singles.tile([1, _P], bf16, tag="lyrow")
            nc.vector.memset(lyrow, _ly)
            # dummy Exp so the activation-table load runs during the DMA wait
            warm = singles.tile([1, 1], f32, tag="warm")
            nc.scalar.activation(out=warm, in_=ones128[0:1, 0:1], func=Act.Exp)

            # ---- input DMA: particle-major tiles
            mstatall = singles.tile([_P, _NT, _D], f32, tag="mstatall")
            nc.sync.dma_start(
                out=mstatall,
                in_=p_in[:].rearrange("(c i) d -> i c d", c=_NT),
            )

            msqall = singles.tile([_P, _NT, _D], f32, tag="msqall")
            nc.vector.tensor_mul(msqall, mstatall, mstatall)

            # ---- small PSUM staging lives in gram-ring slot 0
            # (stats cols 0/2, row-stats 8:16, denf row 20:24, bc 32:36)
            smallA = psG.tile([_P, 512], f32, tag="g", bufs=3, name="smallA")
            st8 = smallA[0:_D, 0:3]
            for c in range(_NT):
                nc.tensor.matmul(
                    st8[:, 0:1], lhsT=mstatall[:, c, :], rhs=onesA,
                    start=(c == 0), stop=(c == _NT - 1),
                    skip_group_check=True,
                )
            for c in range(_NT):
                nc.tensor.matmul(
                    st8[:, 2:3], lhsT=msqall[:, c, :], rhs=onesB,
                    start=(c == 0), stop=(c == _NT - 1),
                    skip_group_check=True,
                )
            # row-form stats (partition 0) for the exp-bias broadcast:
            # the rank-1 rhs needs 1/pilot^2 as a row without a transpose
            strow = smallA[0:1, 8:16]
            for c in range(_NT):
                nc.tensor.matmul(
                    strow[:, 0:_D], lhsT=onesA, rhs=mstatall[:, c, :],
                    start=(c == 0), stop=(c == _NT - 1),
                    skip_group_check=True,
                )
            for c in range(_NT):
                nc.tensor.matmul(
                    strow[:, _D : 2 * _D], lhsT=onesB, rhs=msqall[:, c, :],
                    start=(c == 0), stop=(c == _NT - 1),
                    skip_group_check=True,
                )

            # ---- 8 PE transposes: raw q^T into PSUM [4, 1024]
            # (gram-ring slot 1; reused by gram tile 2)
            psq = psG.tile([_D, _N], f32, tag="g", bufs=3)
            for c in range(_NT):
                nc.tensor.transpose(
                    psq[:, c * _P : (c + 1) * _P], mstatall[:, c, :], ident128
                )

            # ---- var chains: squares on ScalarE (idle), sub/recip on
            # DVE.  Column form (per-partition scales) + row form (rank-1
            # broadcast rhs).
            mm = singles.tile([_D, 1], f32, tag="mm")
            nc.scalar.square(mm, st8[:, 0:1])
            mmr = singles.tile([1, _D], f32, tag="mmr")
            nc.scalar.square(mmr, strow[:, 0:_D])
            denf = singles.tile([_D, 1], f32, tag="denf")
            nc.vector.tensor_sub(denf, st8[:, 2:3], mm)
            phcol2 = singles.tile([_D, 1], f32, tag="phcol2")
            nc.vector.reciprocal(phcol2, denf)
            denfr = singles.tile([1, _D], f32, tag="denfr")
            nc.vector.tensor_sub(denfr, strow[:, _D : 2 * _D], mmr)
            phrow = singles.tile([1, _D], f32, tag="phrow")
            nc.vector.reciprocal(phrow, denfr)

            # ---- scaled weights qsc = q^T * phcol2 on ScalarE (idle
            # until the first exp); halves so qraw can chase
            qsc = singles.tile([_D, _N], fr, tag="qsc")
            nc.scalar.activation(
                out=qsc[:, 0:512], in_=psq[:, 0:512], func=Act.Copy, scale=phcol2
            )
            nc.scalar.activation(
                out=qsc[:, 512:_N], in_=psq[:, 512:_N], func=Act.Copy, scale=phcol2
            )

            # ---- tile-0 Gram weights: qraw slice 0 = qsc[:, 0:128]
            # un-scaled by denf (exact up to the reciprocal rounding)
            qraw = singles.tile([_D, _N], fr, tag="qraw")
            nc.vector.tensor_scalar_mul(qraw[:, 0:_P], qsc[:, 0:_P], denf)

            # ---- 0.5/pilot^2 broadcast [128, 4] via one PE rank-1
            bc_ps = smallA[:, 32:36]
            nc.tensor.matmul(
                bc_ps, lhsT=halfrow, rhs=phrow,
                start=True, stop=True, skip_group_check=True,
            )
            bc_sb = singles.tile([_P, _D], f32, tag="bc_sb")
            nc.vector.tensor_copy(bc_sb, bc_ps)

            # ---- exp bias nhall[:, c] = -r/2
            scr = singles.tile([_P, _NT, _D], f32, tag="scr")
            nc.vector.tensor_mul(
                scr, msqall,
                bc_sb.rearrange("p (o d) -> p o d", o=1).broadcast_to([_P, _NT, _D]),
            )
            nhall = singles.tile([_P, _NT], f32, tag="nhall")
            nc.vector.tensor_reduce(
                out=nhall.rearrange("p (c o) -> p c o", o=1), in_=scr,
                axis=Ax.X, op=Alu.add, negate=True,
            )

            # pilot^2 row for the output staging (smallA dies before the
            # gram ring reuses its slot)
            denf_ps = smallA[0:1, 20:24]
            nc.tensor.matmul(
                denf_ps, lhsT=denf, rhs=ident128[0:_D, 0:_D],
                is_transpose=True, skip_group_check=True,
            )

            # ---- Mp in bf16 (PT-stage weights)
            mtall = singles.tile([_P, _NT, _NM], bf16, tag="mtall")
            nc.vector.memset(mtall[:, :, 0:1], 1.0)
            nc.vector.tensor_copy(mtall[:, :, 1 : 1 + _D], mstatall)
            nc.vector.tensor_copy(mtall[:, :, 1 + _D : _NM], msqall)

            # ---- main loop: upper-triangle row tiles, ascending
            ktall = singles.tile([_P, _KTOT], bf16, tag="ktall")
            pspt = psPT.tile([_NM, _N], f32, tag="pspt")
            pt_sb = singles.tile([_NM, _N + 16], f32, tag="pt_sb")

            def gram(ir):
                lo = _P * ir
                g = psG.tile([_P, _N], f32, tag="g", bufs=3, name=f"g{ir}")
                w = qraw[:, lo : lo + _P]
                if lo < 512:
                    nc.tensor.matmul(
                        g[:, lo:512], lhsT=w, rhs=qsc[:, lo:512],
                        start=True, stop=False, skip_group_check=True,
                    )
                    # -ln2 into the diagonal block (half-weighting); before
                    # chunk B so the chunk-A exp never waits on B
                    nc.tensor.matmul(
                        g[:, lo : lo + _P], lhsT=lnrow, rhs=lyrow,
                        start=False, stop=True, skip_group_check=True,
                    )
                    nc.tensor.matmul(
                        g[:, 512:_N], lhsT=w, rhs=qsc[:, 512:_N],
                        start=True, stop=True, skip_group_check=True,
                    )
                else:
                    nc.tensor.matmul(
                        g[:, lo:_N], lhsT=w, rhs=qsc[:, lo:_N],
                        start=True, stop=False, skip_group_check=True,
                    )
                    nc.tensor.matmul(
                        g[:, lo : lo + _P], lhsT=lnrow, rhs=lyrow,
                        start=False, stop=True, skip_group_check=True,
                    )
                return g

            def exp_tile(ir, g):
                lo = _P * ir
                nc.scalar.activation(
                    out=ktall[:, _OFF[ir] : _OFF[ir] + _C[ir]],
                    in_=g[:, lo:_N],
                    func=Act.Exp, bias=nhall[:, ir : ir + 1],
                )

            def pt_tile(ir):
                lo = _P * ir
                off = _OFF[ir]
                if lo < 512:
                    nc.tensor.matmul(
                        pspt[:, lo:512], lhsT=mtall[:, ir, :],
                        rhs=ktall[:, off : off + (512 - lo)],
                        start=(ir == 0), stop=(ir == 3),
                        skip_group_check=True,
                    )
                    nc.tensor.matmul(
                        pspt[:, 512:_N], lhsT=mtall[:, ir, :],
                        rhs=ktall[:, off + (512 - lo) : off + _C[ir]],
                        start=(ir == 0), stop=(ir == _NT - 1),
                        skip_group_check=True,
                    )
                else:
                    nc.tensor.matmul(
                        pspt[:, lo:_N], lhsT=mtall[:, ir, :],
                        rhs=ktall[:, off : off + _C[ir]],
                        start=False, stop=(ir == _NT - 1),
                        skip_group_check=True,
                    )

            def qraw_slice(k):
                nc.vector.tensor_scalar_mul(
                    qraw[:, _P * k : _P * (k + 1)],
                    qsc[:, _P * k : _P * (k + 1)], denf,
                )

            qraw_slice(1)
            gs = [gram(0), gram(1)]
            nc.vector.tensor_copy(pt_sb[0:1, _N : _N + _D], denf_ps)
            for ir in range(_NT):
                exp_tile(ir, gs[ir])
                if ir + 2 < _NT:
                    qraw_slice(ir + 2)
                    gs.append(gram(ir + 2))
                pt_tile(ir)
                # stage finished PT column blocks out to SBUF
                if ir == 3:
                    nc.vector.tensor_copy(pt_sb[:, 0:512], pspt[:, 0:512])
                elif ir == 5:
                    nc.vector.tensor_copy(pt_sb[:, 512:768], pspt[:, 512:768])
                elif ir == 6:
                    nc.vector.tensor_copy(pt_sb[:, 768:896], pspt[:, 768:896])

            nc.vector.tensor_copy(pt_sb[:, 896:_N], pspt[:, 896:_N])

            nc.sync.dma_start(out=w_out[:], in_=pt_sb)

    nc.compile()
    return nc


def _get_nc():
    global _NC
    if _NC is None:
        _NC = _build_kernel()
    return _NC


def finalize(w, p):
    """Host-side tail.  w [9, 1040]: cols 0:1024 = PT (upper-tri half,
    diag blocks half-weighted, column factor e^{+r_j/2}), row 0 cols
    1024:1028 = pilot^2.  p [1024, 4] = this batch's particles."""
    PT = w[:, 0:_N].astype(np.float64)
    pilot2 = w[0, _N : _N + _D].astype(np.float64)
    p = p.astype(np.float64)
    r = (p * p / pilot2[None, :]).sum(axis=1)
    cx = np.exp(-0.5 * r)
    Mp = np.concatenate([np.ones((_N, 1)), p, p * p], axis=1)
    W = PT @ (Mp * cx[:, None])  # [9, 9]
    d = np.arange(_D)
    g = 2.0 * (W[0, 5 + d] + W[5 + d, 0] - 2.0 * W[1 + d, 1 + d])
    v00 = 2.0 * W[0, 0]
    s2 = (g / pilot2 - v00) * _INV_SQRT_2PI
    denom = _N * (_N - 1)
    pilot5 = pilot2**2 * np.sqrt(pilot2)
    I2 = s2 / pilot5 / denom
    J1 = _RK / I2
    base = J1 / _N
    return (np.sign(base) * np.abs(base) ** 0.2).astype(np.float32)


def kernel(particles, weights=None, **_unused):
    from concourse.bass_utils import run_bass_kernel_spmd

    particles = np.ascontiguousarray(np.asarray(particles), dtype=np.float32)
    assert particles.shape == (_B, _N, _D), particles.shape

    nc = _get_nc()
    in_maps = [{"p": particles[c]} for c in range(_B)]
    res = run_bass_kernel_spmd(nc, in_maps, list(range(_B)))

    out = np.empty((_B, _D), np.float32)
    for c in range(_B):
        out[c] = finalize(res.results[c]["wout"], particles[c])
    return out


# revision 36
# speedup vs baseline: 1.0507x; 1.0114x over previous
"""Trainium2 Bass kernel for nn_BandwidthPredictorNNHall.

Math: for each batch b (8 of them, one per NeuronCore) with particles
x [n=1024, d=4]:
    pilot_d = 1.0592 * std(x_d, ddof=1) * n^(-1/8)
    q = x / pilot,   K_ij = exp(-0.5 * |q_i - q_j|^2)
    s2_d = sum_ij K_ij ((q_jd - q_id)^2 - 1)
    s3 terms are exactly 0 by antisymmetry -> bandwidth2 treated as 0.

K is symmetric, so only the upper-triangular block half is computed:
row tile ir (128 rows) covers columns [128*ir, 1024).  Diagonal blocks
are half-weighted by accumulating -ln2 into their Gram PSUM region (a
tiny bf16 rank-1 matmul) so that, with
    PT[a, j] = sum_{i in tiles <= tile(j)} Mp[i, a] K''_ij
(K'' = K * e^{r_j/2}, the column factor from the bias-only exp), the
host gets W = PT @ (Mp * e^{-r/2}) and V = W + W^T == Mp^T K Mp
exactly.  The device ships PT [9, 1024] and pilot^2; the ~0.6 Mflop W
assembly and the final ~30 scalar flops run on the host.

Device pipeline per core:
  - One DMA in: particle-major [128, 8(tile), 4].
  - Stats on PE ([4, {1,2}] PSUM columns, pre-scaled ones vectors so the
    DVE var chain is copy/mult/sub/reciprocal -> phcol2 = 1/pilot^2,
    denf = pilot^2).
  - 8 PE transposes build raw q^T in PSUM; ScalarE Copy activations with
    per-partition scale phcol2 produce the scaled Gram weights qsc; the
    raw moving operand qraw = qsc un-scaled by denf (one DVE
    tensor_scalar per half; f32r-rounded producers as fp32r requires).
  - exp bias -r/2: PE transpose + rank-1 broadcast of 0.5/pilot^2,
    Pool multiply, DVE reduce.
  - Main loop (ascending tiles): Gram chunks (f32r, 1 cycle/col) ->
    one Exp per tile (bias -r_i/2) -> bf16 KT -> PT chunks (bf16)
    accumulating into a nested [9, 1024] PSUM region.  PSUM column
    block ir is final right after tile ir, so PT is staged out to SBUF
    in three pieces (after tiles 3, 6, 7) and the tail is just the last
    [9, 128] copy + one DMA.
"""

import sys

sys.path.insert(0, "/opt/trn_rl_repo")

import numpy as np

_B, _N, _D = 8, 1024, 4
_P = 128
_NT = _N // _P  # 8 row tiles
_NM = 1 + 2 * _D  # 9 basis columns: [1, p, p^2]
_INV_SQRT_2PI = 1.0 / np.sqrt(2.0 * np.pi)
_RK = 0.282095
_FACT = 1.0592 * float(_N) ** (-1.0 / (4 + _D))

# column extents per row tile (upper triangle) and KT storage offsets
_C = [(_N - _P * ir) for ir in range(_NT)]
_OFF = [sum(_C[:ir]) for ir in range(_NT)]
_KTOT = sum(_C)  # 4608

_NC = None  # compiled Bass module cache


def _build_kernel():
    import concourse.bass as bass  # noqa: F401
    import concourse.tile as tile
    from concourse import bacc, mybir
    from concourse.masks import make_identity

    f32 = mybir.dt.float32
    fr = mybir.dt.float32r
    bf16 = mybir.dt.bfloat16
    Act = mybir.ActivationFunctionType
    Alu = mybir.AluOpType
    Ax = mybir.AxisListType

    # split -ln2 across the two bf16 rank-1 factors so the product is
    # ln2 to ~2^-16 relative
    import ml_dtypes

    _lx = float(np.abs(np.sqrt(np.log(2.0))).astype(ml_dtypes.bfloat16))
    _ly = float(np.array(np.log(2.0) / _lx, np.float32).astype(ml_dtypes.bfloat16))

    nc = bacc.Bacc("TRN2", target_bir_lowering=False, debug=False, num_devices=_B)
    p_in = nc.dram_tensor("p", [_N, _D], f32, kind="ExternalInput")
    w_out = nc.dram_tensor("wout", [_NM, _N + 16], f32, kind="ExternalOutput")

    with tile.TileContext(nc) as tc:
        with (
            tc.tile_pool(name="singles", bufs=1) as singles,
            tc.tile_pool(name="psG", bufs=1, space="PSUM") as psG,
            tc.tile_pool(name="psPT", bufs=1, space="PSUM") as psPT,
        ):
            ident128 = singles.tile([_P, _P], f32, tag="identf")
            make_identity(nc, ident128)
            ones128 = singles.tile([_P, 1], f32, tag="ones128")
            nc.vector.memset(ones128, 1.0)
            # pre-scaled ones so denf comes out as pilot^2 directly:
            #   m = (sum p * a)^2, denf = sum p^2 * b - m
            bconst = _FACT * _FACT / (_N - 1)
            aconst = _FACT / np.sqrt(float(_N) * (_N - 1))
            onesA = singles.tile([_P, 1], f32, tag="onesA")
            nc.vector.memset(onesA, aconst)
            onesB = singles.tile([_P, 1], f32, tag="onesB")
            nc.vector.memset(onesB, bconst)
            halfrow = singles.tile([1, _P], f32, tag="halfrow")
            nc.vector.memset(halfrow, 0.5)
            # bf16 rank-1 factors for the -ln2 diagonal half-weighting
            lnrow = singles.tile([1, _P], bf16, tag="lnrow")
            nc.vector.memset(lnrow, -_lx)
            lyrow = singles.tile([1, _P], bf16, tag="lyrow")
            nc.vector.memset(lyrow, _ly)
            # dummy Exp so the activation-table load runs during the DMA wait
            warm = singles.tile([1, 1], f32, tag="warm")
            nc.scalar.activation(out=warm, in_=ones128[0:1, 0:1], func=Act.Exp)

            # ---- input DMA: particle-major tiles
            mstatall = singles.tile([_P, _NT, _D], f32, tag="mstatall")
            nc.sync.dma_start(
                out=mstatall,
                in_=p_in[:].rearrange("(c i) d -> i c d", c=_NT),
            )

            msqall = singles.tile([_P, _NT, _D], f32, tag="msqall")
            nc.vector.tensor_mul(msqall, mstatall, mstatall)

            # ---- small PSUM staging lives in gram-ring slot 0
            # (stats cols 0/2, row-stats 8:16, denf row 20:24, bc 32:36)
            smallA = psG.tile([_P, 512], f32, tag="g", bufs=3, name="smallA")
            st8 = smallA[0:_D, 0:3]
            for c in range(_NT):
                nc.tensor.matmul(
                    st8[:, 0:1], lhsT=mstatall[:, c, :], rhs=onesA,
                    start=(c == 0), stop=(c == _NT - 1),
                    skip_group_check=True,
                )
            for c in range(_NT):
                nc.tensor.matmul(
                    st8[:, 2:3], lhsT=msqall[:, c, :], rhs=onesB,
                    start=(c == 0), stop=(c == _NT - 1),
                    skip_group_check=True,
                )
            # row-form stats (partition 0) for the exp-bias broadcast:
            # the rank-1 rhs needs 1/pilot^2 as a row without a transpose
            strow = smallA[0:1, 8:16]
            for c in range(_NT):
                nc.tensor.matmul(
                    strow[:, 0:_D], lhsT=onesA, rhs=mstatall[:, c, :],
                    start=(c == 0), stop=(c == _NT - 1),
                    skip_group_check=True,
                )
            for c in range(_NT):
                nc.tensor.matmul(
                    strow[:, _D : 2 * _D], lhsT=onesB, rhs=msqall[:, c, :],
                    start=(c == 0), stop=(c == _NT - 1),
                    skip_group_check=True,
                )

            # ---- 8 PE transposes: raw q^T into PSUM [4, 1024]
            # (gram-ring slot 1; reused by gram tile 2)
            psq = psG.tile([_D, _N], f32, tag="g", bufs=3)
            for c in range(_NT):
                nc.tensor.transpose(
                    psq[:, c * _P : (c + 1) * _P], mstatall[:, c, :], ident128
                )

            # ---- var chains: squares on ScalarE (idle), sub/recip on
            # DVE.  Column form (per-partition scales) + row form (rank-1
            # broadcast rhs).
            mm = singles.tile([_D, 1], f32, tag="mm")
            nc.scalar.square(mm, st8[:, 0:1])
            mmr = singles.tile([1, _D], f32, tag="mmr")
            nc.scalar.square(mmr, strow[:, 0:_D])
            denf = singles.tile([_D, 1], f32, tag="denf")
            nc.vector.tensor_sub(denf, st8[:, 2:3], mm)
            phcol2 = singles.tile([_D, 1], f32, tag="phcol2")
            nc.vector.reciprocal(phcol2, denf)
            denfr = singles.tile([1, _D], f32, tag="denfr")
            nc.vector.tensor_sub(denfr, strow[:, _D : 2 * _D], mmr)
            phrow = singles.tile([1, _D], f32, tag="phrow")
            nc.vector.reciprocal(phrow, denfr)

            # ---- scaled weights qsc = q^T * phcol2 on ScalarE (idle
            # until the first exp); halves so qraw can chase
            qsc = singles.tile([_D, _N], fr, tag="qsc")
            nc.scalar.activation(
                out=qsc[:, 0:512], in_=psq[:, 0:512], func=Act.Copy, scale=phcol2
            )
            nc.scalar.activation(
                out=qsc[:, 512:_N], in_=psq[:, 512:_N], func=Act.Copy, scale=phcol2
            )

            # ---- tile-0 Gram weights: qraw slice 0 = qsc[:, 0:128]
            # un-scaled by denf (exact up to the reciprocal rounding)
            qraw = singles.tile([_D, _N], fr, tag="qraw")
            nc.vector.tensor_scalar_mul(qraw[:, 0:_P], qsc[:, 0:_P], denf)

            # ---- 0.5/pilot^2 broadcast [128, 4] via one PE rank-1
            bc_ps = smallA[:, 32:36]
            nc.tensor.matmul(
                bc_ps, lhsT=halfrow, rhs=phrow,
                start=True, stop=True, skip_group_check=True,
            )
            bc_sb = singles.tile([_P, _D], f32, tag="bc_sb")
            nc.vector.tensor_copy(bc_sb, bc_ps)

            # ---- exp bias nhall[:, c] = -r/2
            scr = singles.tile([_P, _NT, _D], f32, tag="scr")
            nc.vector.tensor_mul(
                scr, msqall,
                bc_sb.rearrange("p (o d) -> p o d", o=1).broadcast_to([_P, _NT, _D]),
            )
            nhall = singles.tile([_P, _NT], f32, tag="nhall")
            nc.vector.tensor_reduce(
                out=nhall.rearrange("p (c o) -> p c o", o=1), in_=scr,
                axis=Ax.X, op=Alu.add, negate=True,
            )

            # pilot^2 row for the output staging (smallA dies before the
            # gram ring reuses its slot)
            denf_ps = smallA[0:1, 20:24]
            nc.tensor.matmul(
                denf_ps, lhsT=denf, rhs=ident128[0:_D, 0:_D],
                is_transpose=True, skip_group_check=True,
            )

            # ---- Mp in bf16 (PT-stage weights)
            mtall = singles.tile([_P, _NT, _NM], bf16, tag="mtall")
            nc.vector.memset(mtall[:, :, 0:1], 1.0)
            nc.vector.tensor_copy(mtall[:, :, 1 : 1 + _D], mstatall)
            nc.vector.tensor_copy(mtall[:, :, 1 + _D : _NM], msqall)

            # ---- main loop: upper-triangle row tiles, ascending
            ktall = singles.tile([_P, _KTOT], bf16, tag="ktall")
            pspt = psPT.tile([_NM, _N], f32, tag="pspt")
            pt_sb = singles.tile([_NM, _N + 16], f32, tag="pt_sb")

            def gram(ir):
                lo = _P * ir
                g = psG.tile([_P, _N], f32, tag="g", bufs=3, name=f"g{ir}")
                w = qraw[:, lo : lo + _P]
                if lo < 512:
                    nc.tensor.matmul(
                        g[:, lo:512], lhsT=w, rhs=qsc[:, lo:512],
                        start=True, stop=False, skip_group_check=True,
                    )
                    # -ln2 into the diagonal block (half-weighting); before
                    # chunk B so the chunk-A exp never waits on B
                    nc.tensor.matmul(
                        g[:, lo : lo + _P], lhsT=lnrow, rhs=lyrow,
                        start=False, stop=True, skip_group_check=True,
                    )
                    nc.tensor.matmul(
                        g[:, 512:_N], lhsT=w, rhs=qsc[:, 512:_N],
                        start=True, stop=True, skip_group_check=True,
                    )
                else:
                    nc.tensor.matmul(
                        g[:, lo:_N], lhsT=w, rhs=qsc[:, lo:_N],
                        start=True, stop=False, skip_group_check=True,
                    )
                    nc.tensor.matmul(
                        g[:, lo : lo + _P], lhsT=lnrow, rhs=lyrow,
                        start=False, stop=True, skip_group_check=True,
                    )
                return g

            def exp_tile(ir, g):
                lo = _P * ir
                nc.scalar.activation(
                    out=ktall[:, _OFF[ir] : _OFF[ir] + _C[ir]],
                    in_=g[:, lo:_N],
                    func=Act.Exp, bias=nhall[:, ir : ir + 1],
                )

            def pt_tile(ir):
                lo = _P * ir
                off = _OFF[ir]
                if lo < 512:
                    nc.tensor.matmul(
                        pspt[:, lo:512], lhsT=mtall[:, ir, :],
                        rhs=ktall[:, off : off + (512 - lo)],
                        start=(ir == 0), stop=(ir == 3),
                        skip_group_check=True,
                    )
                    nc.tensor.matmul(
                        pspt[:, 512:_N], lhsT=mtall[:, ir, :],
                        rhs=ktall[:, off + (512 - lo) : off + _C[ir]],
                        start=(ir == 0), stop=(ir == _NT - 1),
                        skip_group_check=True,
                    )
                else:
                    nc.tensor.matmul(
                        pspt[:, lo:_N], lhsT=mtall[:, ir, :],
                        rhs=ktall[:, off : off + _C[ir]],
                        start=False, stop=(ir == _NT - 1),
                        skip_group_check=True,
                    )

            def qraw_slice(k):
                nc.vector.tensor_scalar_mul(
                    qraw[:, _P * k : _P * (k + 1)],
                    qsc[:, _P * k : _P * (k + 1)], denf,
                )

            qraw_slice(1)
            gs = [gram(0), gram(1)]
            nc.vector.tensor_copy(pt_sb[0:1, _N : _N + _D], denf_ps)
            for ir in range(_NT):
                exp_tile(ir, gs[ir])
                if ir + 2 < _NT:
                    qraw_slice(ir + 2)
                    gs.append(gram(ir + 2))
                pt_tile(ir)
                # stage the finished lower PT half out to SBUF mid-loop
                if ir == 3:
                    nc.vector.tensor_copy(pt_sb[:, 0:512], pspt[:, 0:512])

            # upper half after the last PT: ScalarE and DVE halves run
            # concurrently (both engines are idle by now)
            nc.scalar.copy(pt_sb[:, 512:768], pspt[:, 512:768])
            nc.vector.tensor_copy(pt_sb[:, 768:_N], pspt[:, 768:_N])

            nc.sync.dma_start(out=w_out[:], in_=pt_sb)

    nc.compile()
    return nc


def _get_nc():
    global _NC
    if _NC is None:
        _NC = _build_kernel()
    return _NC


def finalize(w, p):
    """Host-side tail.  w [9, 1040]: cols 0:1024 = PT (upper-tri half,
    diag blocks half-weighted, column factor e^{+r_j/2}), row 0 cols
    1024:1028 = pilot^2.  p [1024, 4] = this batch's particles."""
    PT = w[:, 0:_N].astype(np.float64)
    pilot2 = w[0, _N : _N + _D].astype(np.float64)
    p = p.astype(np.float64)
    r = (p * p / pilot2[None, :]).sum(axis=1)
    cx = np.exp(-0.5 * r)
    Mp = np.concatenate([np.ones((_N, 1)), p, p * p], axis=1)
    W = PT @ (Mp * cx[:, None])  # [9, 9]
    d = np.arange(_D)
    g = 2.0 * (W[0, 5 + d] + W[5 + d, 0] - 2.0 * W[1 + d, 1 + d])
    v00 = 2.0 * W[0, 0]
    s2 = (g / pilot2 - v00) * _INV_SQRT_2PI
    denom = _N * (_N - 1)
    pilot5 = pilot2**2 * np.sqrt(pilot2)
    I2 = s2 / pilot5 / denom
    J1 = _RK / I2
    base = J1 / _N
    return (np.sign(base) * np.abs(base) ** 0.2).astype(np.float32)


def kernel(particles, weights=None, **_unused):
    from concourse.bass_utils import run_bass_kernel_spmd

    particles = np.ascontiguousarray(np.asarray(particles), dtype=np.float32)
    assert particles.shape == (_B, _N, _D), particles.shape

    nc = _get_nc()
    in_maps = [{"p": particles[c]} for c in range(_B)]
    res = run_bass_kernel_spmd(nc, in_maps, list(range(_B)))

    out = np.empty((_B, _D), np.float32)
    for c in range(_B):
        out[c] = finalize(res.results[c]["wout"], particles[c])
    return out
